# revision 1
# baseline (speedup 1.0000x reference)
"""Trainium2 Bass kernel for nn_DecoderTopDown (top-down attention LSTM decoder).

Strategy (8 NeuronCores, tensor-parallel over gate/hidden dims):
  - Each core owns a 128-wide slice of H1 and H2 (gates reordered [i|f|o|g]).
  - Per step: gate GEMMs in [M=batch, N=gates] layout; recurrent h exchanged via
    ragged-width AllGathers (only active batch columns).
  - Attention: R=36 split across cores (5/5/5/5/4/4/4/4 padded to 5); logits
    exchanged via a tiny AllGather; softmax done UNNORMALIZED: exp in [RT,B]
    layout feeds the einsum directly (no transposes), the 1/sum is applied
    after the einsum via a per-partition scalar.
  - Einsum over regions as block-diagonal matmuls, 2 batches per matmul
    (blocks at partitions 0/64 - engine partition access must be 32-aligned),
    against precomputed VW2 = Vmat @ W2v.T pair tiles, accumulated into a
    separate PSUM and normalized afterwards (unnormalized-softmax trick).
  - LSTM cell in 2 ACT + 4 fused scalar_tensor_tensor ops using doubled state
    conventions C=2c, H=2h (weight scales compensated on host) and g-gate rows
    pre-doubled so one tanh(x*0.5) serves all four gates.
  - Vocab projection (Wl) computed INSIDE the loop (per step, vocab-sharded)
    in the PE-idle windows of the collectives; pre1[t] (embedding+static input
    terms) also computed in-loop two steps ahead. Keeps the PE continuously
    busy (p-state ramp) and removes the pre/post phases' serial time.

kernel(**inputs) takes FULL inputs, returns FULL [B, T, VOC] float32 output.
"""
import sys, os
sys.path.insert(0, "/opt/trn_rl_repo")

import numpy as np
import ml_dtypes

BF16 = ml_dtypes.bfloat16

# Problem dims (hardcoded per contest rules)
B, R, T = 128, 36, 40
E, V, H1, H2, PH, VOC = 1024, 2048, 1024, 1024, 256, 10000
NC_ = 8                    # cores
GS = 4 * H1 // NC_         # per-core gate slice = 512
HS = H1 // NC_             # per-core hidden slice = 128
RP = 5                     # padded r's per core
RT = NC_ * RP              # padded total r rows = 40
VS = VOC // NC_            # vocab slice = 1250
KT1 = H1 // 128            # 8 k-tiles for H-sized contractions
KTV = V // 128             # 16 k-tiles for V-sized contractions
NPAIR = B // 2             # 64 block-diag pairs (blocks at partitions 0 and 64)
BDK = 64 + RT              # 104: rows [0,40) = even b, [64,104) = odd b

_cache = {}
LAST_NC = None
LAST_IN_MAPS = None


def _r_assign():
    """r-split across cores: cores 0-3 get 5, cores 4-7 get 4 (+1 pad).
    Returns per-core lists of global r (or -1 for pad) and the row->r map."""
    per_core = []
    row_r = []
    nxt = 0
    for c in range(NC_):
        cnt = 5 if c < 4 else 4
        rs = list(range(nxt, nxt + cnt)) + [-1] * (RP - cnt)
        nxt += cnt
        per_core.append(rs)
        row_r += rs
    assert nxt == R
    return per_core, row_r   # row_r: length 40, -1 = pad


def _prep_inputs(inputs):
    """Host-side prep: gather embeddings, transpose/slice/scale weights per core.

    Scaling conventions baked into the weights:
      - gate rows reordered [i|f|o|g]; g rows DOUBLED so one tanh(x*0.5) gives
        tanh(x/2) for i,f,o (sigmoid building block) and tanh(x) for g.
      - hidden states stored doubled (H=2h): every weight consuming h gets 0.5.
      - bha folded into Va's bias.
    """
    Vmat = inputs["Vmat"].astype(np.float32)
    uv = inputs["union_vfeats"].astype(np.float32)
    captions = inputs["captions"]
    lengths = np.asarray(inputs["lengths"]).astype(np.int64)
    embW = inputs["embed_W"].astype(np.float32)
    Wi1, Wh1, b1 = inputs["Wi1"], inputs["Wh1"], inputs["b1"]
    Wi2, Wh2, b2 = inputs["Wi2"], inputs["Wh2"], inputs["b2"]
    Wva, bva = inputs["Wva"], inputs["bva"]
    Wha, bha = inputs["Wha"], inputs["bha"]
    wa = inputs["wa"]
    Wl, bl = inputs["Wl"], inputs["bl"]

    per_core_r, row_r = _r_assign()

    # active counts per step (lengths sorted descending)
    n_t = [int((lengths > t).sum()) for t in range(T)]

    emb = embW[captions]                              # [B,T,E]
    embT = np.ascontiguousarray(emb.transpose(1, 2, 0)).astype(BF16)   # [T,E,B]
    uvT = np.ascontiguousarray(uv.T).astype(BF16)     # [V,B]
    VmatT = np.ascontiguousarray(Vmat.transpose(2, 0, 1))  # [V,B,R] fp32

    WhaT = np.ascontiguousarray(Wha.T * 0.5).astype(BF16)  # [H1,PH] (h comp)
    wab = np.broadcast_to(wa[None, :], (B, PH)).astype(BF16).copy()
    bvab = np.broadcast_to((bva + bha)[None, :], (B, PH)).astype(np.float32).copy()

    # VmatT3: block-diag einsum rhs source [V, NPAIR, 104] (zeros at pad-r rows)
    VmatT3 = np.zeros((V, NPAIR, BDK), dtype=np.float32)
    for c2 in range(2):
        bs = 2 * np.arange(NPAIR) + c2
        for jj, r in enumerate(row_r):
            if r >= 0:
                VmatT3[:, :, 64 * c2 + jj] = VmatT[:, bs, r]
    VmatT3 = VmatT3.astype(BF16)

    gscale = np.ones(GS, np.float32)
    gscale[3 * HS:] = 2.0                             # g-gate rows doubled

    in_maps = []
    for c in range(NC_):
        # gate rows, reordered [i|f|o|g]
        def perm(Hn):
            base = np.arange(HS * c, HS * (c + 1))
            return np.concatenate([base, base + Hn, base + 3 * Hn, base + 2 * Hn])
        p1 = perm(H1); p2 = perm(H2)

        W1h2T = np.ascontiguousarray(Wi1[p1, 0:H2].T * gscale * 0.5).astype(BF16)
        Wh1T = np.ascontiguousarray(Wh1[p1, :].T * gscale * 0.5).astype(BF16)
        W1eT = np.ascontiguousarray(Wi1[p1, H2 + V:].T * gscale).astype(BF16)
        W1vT = np.ascontiguousarray(Wi1[p1, H2:H2 + V].T * gscale).astype(BF16)
        W2h1T = np.ascontiguousarray(Wi2[p2, V:].T * gscale * 0.5).astype(BF16)
        Wh2T = np.ascontiguousarray(Wh2[p2, :].T * gscale * 0.5).astype(BF16)
        W2vT = np.ascontiguousarray(Wi2[p2, 0:V].T * gscale).astype(BF16)
        b1b = np.broadcast_to((b1[p1] * gscale)[None, :], (B, GS)).astype(np.float32).copy()
        b2b = np.broadcast_to((b2[p2] * gscale)[None, :], (B, GS)).astype(np.float32).copy()

        vs = slice(VS * c, VS * (c + 1))
        WlT = np.ascontiguousarray(Wl[vs, :].T * 0.5).astype(BF16)        # [1024,1250]
        blb = np.broadcast_to(bl[vs][None, :], (B, VS)).astype(np.float32).copy()

        # attention r-slice: VmatTr [V, RP, B], zeros for pad
        rs = per_core_r[c]
        VmatTr = np.zeros((V, RP, B), dtype=np.float32)
        for j, r in enumerate(rs):
            if r >= 0:
                VmatTr[:, j, :] = VmatT[:, :, r]
        VmatTr = VmatTr.astype(BF16)
        lmask = np.array([[1.0] if r >= 0 else [0.0] for r in rs], np.float32)
        loff = np.array([[0.0] if r >= 0 else [-30.0] for r in rs], np.float32)

        in_maps.append(dict(
            embT=embT, uvT=uvT, VmatT3=VmatT3, VmatTr=VmatTr,
            W1h2T=W1h2T, Wh1T=Wh1T, W1eT=W1eT, W1vT=W1vT,
            W2h1T=W2h1T, Wh2T=Wh2T, W2vT=W2vT,
            WhaT=WhaT, wab=wab, bvab=bvab,
            WlT=WlT, blb=blb, b1b=b1b, b2b=b2b,
            lmask=lmask, loff=loff,
        ))
    return in_maps, n_t


def _build(n_t, Tsteps):
    """Build + compile the Bass program (lengths-specialized)."""
    from concourse import bass, bacc, tile, mybir, masks

    f32 = mybir.dt.float32
    bf16 = mybir.dt.bfloat16
    f8 = mybir.dt.float8e4
    DR = mybir.MatmulPerfMode.DoubleRow
    AT = mybir.ActivationFunctionType
    OP = mybir.AluOpType

    nc = bacc.Bacc("TRN2", target_bir_lowering=False, debug=False, num_devices=NC_)

    # ---------------- I/O declarations ----------------
    def din(name, shape, dt=bf16):
        return nc.dram_tensor(name, shape, dt, kind="ExternalInput")

    embT = din("embT", [T, E, B])
    uvT = din("uvT", [V, B])
    VmatT3 = din("VmatT3", [V, NPAIR, BDK])
    VmatTr = din("VmatTr", [V, RP, B])
    W1h2T = din("W1h2T", [H2, GS]); Wh1T = din("Wh1T", [H1, GS])
    W1eT = din("W1eT", [E, GS]); W1vT = din("W1vT", [V, GS])
    W2h1T = din("W2h1T", [H1, GS]); Wh2T = din("Wh2T", [H2, GS])
    W2vT = din("W2vT", [V, GS])
    WhaT = din("WhaT", [H1, PH])
    wab = din("wab", [B, PH])
    bvab = din("bvab", [B, PH], f32)
    WlT = din("WlT", [H2, VS])
    blb = din("blb", [B, VS], f32)
    b1b = din("b1b", [B, GS], f32); b2b = din("b2b", [B, GS], f32)
    lmask = din("lmask", [RP, 1], f32); loff = din("loff", [RP, 1], f32)

    out = nc.dram_tensor("out", [B, T, VS], f32, kind="ExternalOutput")

    RG = [list(range(NC_))]
    NVT = (VS + 511) // 512          # 3 vocab chunks per step

    with tile.TileContext(nc) as tc:
      with (
        tc.tile_pool(name="persist", bufs=1) as P,
        tc.tile_pool(name="loopres", bufs=1) as LP,
        tc.tile_pool(name="bounce", bufs=3, space="DRAM") as BP,
      ):
        # ---------- persistent SBUF state ----------
        ident = P.tile([128, 128], bf16)
        masks.make_identity(nc, ident[:])
        identf = P.tile([128, 128], f32)
        masks.make_identity(nc, identf[:])

        h1T = P.tile([128, KT1, B], bf16)     # gathered (2*h1)^T  [feat, b]
        h2T = P.tile([128, KT1, B], bf16)
        h1n = P.tile([B, HS], bf16)           # own slice, [b, feat], = 2*h1
        h2n = P.tile([B, HS], bf16)
        c1 = P.tile([B, HS], f32)             # = 2*c1
        c2 = P.tile([B, HS], f32)
        for tbuf in (h1T, h2T, h1n, h2n, c1, c2):
            nc.vector.memset(tbuf[:], 0.0)

        atten_bd = LP.tile([128, NPAIR * 128], bf16)  # block-diag lhsT (zeros persist)
        nc.vector.memset(atten_bd[:], 0.0)
        ones_r = P.tile([RT, 1], bf16)
        nc.vector.memset(ones_r[:], 1.0)

        VW2 = LP.tile([128, NPAIR, GS], bf16)  # einsum rhs pairs (104 rows used)
        Va = LP.tile([B, RP, PH], bf16)        # attention bias term (incl bva+bha)

        # loop-resident weights (allocated here; loaded after Va below so the
        # pre-phase's immediately-needed DMAs go first in the queue)
        w1h2 = LP.tile([128, KT1, GS], bf16)
        wh1 = LP.tile([128, KT1, GS], bf16)
        w2h1 = LP.tile([128, KT1, GS], bf16)
        wh2 = LP.tile([128, KT1, GS], bf16)
        wha = LP.tile([128, KT1, PH], bf16)
        wl = LP.tile([128, KT1, VS], bf16)
        b2s = P.tile([B, GS], f32)
        was = P.tile([B, PH], bf16)
        bls = P.tile([B, VS], f32)
        lmasks = P.tile([RP, 1], f32)
        loffs = P.tile([RP, 1], f32)

        # pre1 ring (SBUF) + embT prefetch ring
        pre_ring = [LP.tile([B, GS], bf16, name=f"pre{i}") for i in range(3)]
        ebr = [LP.tile([128, KT1, B], bf16, name=f"ebr{i}") for i in range(3)]

        # ================= PRECOMPUTE =================
        base1 = LP.tile([B, GS], f32)
        with (
            tc.tile_pool(name="pre_sb", bufs=3) as PS,
            tc.tile_pool(name="pre_ps", bufs=2, space="PSUM") as PP,
            tc.tile_pool(name="pre_psg", bufs=1, space="PSUM") as PPG,
        ):
            # --- base1 = uv @ W1v.T + b1 ---
            with tc.tile_pool(name="pre_w1", bufs=1) as PW:
                b1s = PW.tile([B, GS], f32); nc.sync.dma_start(b1s[:], b1b[:])
                w1v = PW.tile([128, KTV, GS], bf16)
                nc.sync.dma_start(w1v[:], W1vT.ap().rearrange("(k p) n -> p k n", p=128))
                uvs = PW.tile([128, KTV, B], bf16)
                nc.sync.dma_start(uvs[:], uvT.ap().rearrange("(k p) n -> p k n", p=128))
                ps0 = PP.tile([B, GS], f32, tag="pre")
                for k in range(KTV):
                    nc.tensor.matmul(ps0[:], uvs[:, k, :], w1v[:, k, :],
                                     start=(k == 0), stop=(k == KTV - 1))
                nc.vector.tensor_tensor(out=base1[:], in0=ps0[:], in1=b1s[:], op=OP.add)

            # --- w1e stays resident for in-loop pre1 ---
            w1e = LP.tile([128, KT1, GS], bf16)
            nc.sync.dma_start(w1e[:], W1eT.ap().rearrange("(k p) n -> p k n", p=128))

            def pre1_compute(t, psum_pool, psum_tag):
                """pre_ring[t%3] = embT[t] @ W1e.T + base1 (ebr[t%3] preloaded)."""
                pps = psum_pool.tile([B, GS], f32, tag=psum_tag)
                for k in range(KT1):
                    nc.tensor.matmul(pps[:], ebr[t % 3][:, k, :], w1e[:, k, :],
                                     start=(k == 0), stop=(k == KT1 - 1))
                nc.vector.tensor_tensor(out=pre_ring[t % 3][:], in0=pps[:],
                                        in1=base1[:], op=OP.add)

            def emb_load(t):
                nc.sync.dma_start(
                    ebr[t % 3][:], embT.ap()[t].rearrange("(k p) n -> p k n", p=128))

            # bootstrap pre1[0], pre1[1]
            for t in range(min(2, Tsteps)):
                emb_load(t)
                pre1_compute(t, PP, "pre")

            # --- Va[b, j, ph] = (Vmat @ Wva.T + bva + bha) for own r's ---
            with tc.tile_pool(name="pre_w3", bufs=1) as PW:
                bvas = PW.tile([B, PH], f32); nc.sync.dma_start(bvas[:], bvab[:])
                wva = PW.tile([128, KTV, PH], bf16)
                nc.sync.dma_start(
                    wva[:], nc.dram_tensor("WvaT", [V, PH], bf16, kind="ExternalInput")
                    .ap().rearrange("(k p) n -> p k n", p=128))
                for j in range(RP):
                    vps = PP.tile([B, GS], f32, tag="pre")
                    for k in range(KTV):
                        vtr = PS.tile([128, B], bf16, tag="vtr")
                        nc.sync.dma_start(
                            vtr[:], VmatTr.ap().rearrange("(k p) j n -> k p j n", p=128)[k, :, j])
                        nc.tensor.matmul(vps[:, 0:PH], vtr[:], wva[:, k, :],
                                         start=(k == 0), stop=(k == KTV - 1))
                    nc.vector.tensor_tensor(out=Va[:, j, :], in0=vps[:, 0:PH], in1=bvas[:], op=OP.add)

            # loop-resident weight loads (overlap VW2 compute)
            for dst, s_ in ((w1h2, W1h2T), (wh1, Wh1T), (w2h1, W2h1T),
                            (wh2, Wh2T), (wha, WhaT), (wl, WlT)):
                nc.sync.dma_start(dst[:], s_.ap().rearrange("(k p) n -> p k n", p=128))
            nc.sync.dma_start(b2s[:], b2b[:])
            nc.sync.dma_start(was[:], wab[:])
            nc.sync.dma_start(bls[:], blb[:])
            nc.sync.dma_start(lmasks[:], lmask[:])
            nc.sync.dma_start(loffs[:], loff[:])

            # --- VW2 pair tiles: VmatT3 @ W2v.T -> [104, GS] bf16 each ---
            with tc.tile_pool(name="pre_w4", bufs=1) as PW:
                w2v = PW.tile([128, KTV, GS], bf16)
                nc.sync.dma_start(w2v[:], W2vT.ap().rearrange("(k p) n -> p k n", p=128))
                GRP = 6
                for i0 in range(0, NPAIR, GRP):
                    cnt = min(GRP, NPAIR - i0)
                    pss = [PPG.tile([BDK, GS], f32, tag=f"vw{g}", name=f"vw{g}") for g in range(cnt)]
                    for k in range(KTV):
                        v3 = PS.tile([128, GRP * BDK], bf16, tag="v3")
                        nc.sync.dma_start(
                            v3[:, 0:cnt * BDK],
                            VmatT3.ap().rearrange("(k p) i n -> k p i n", p=128)[k, :, i0:i0 + cnt]
                        )
                        for g in range(cnt):
                            nc.tensor.matmul(pss[g][:], v3[:, g * BDK:(g + 1) * BDK],
                                             w2v[:, k, :], start=(k == 0), stop=(k == KTV - 1))
                    for g in range(cnt):
                        nc.scalar.copy(VW2[0:BDK, i0 + g, :], pss[g][:])

        # ================= RECURRENT LOOP =================
        with (
            tc.tile_pool(name="work", bufs=1) as W,
            tc.tile_pool(name="owork", bufs=2) as OW,
            tc.tile_pool(name="pg1", bufs=2, space="PSUM") as PG1,
            tc.tile_pool(name="pg2", bufs=1, space="PSUM") as PG2,
            tc.tile_pool(name="pse", bufs=1, space="PSUM") as PSE,
            tc.tile_pool(name="psm", bufs=2, space="PSUM") as PSM,
            tc.tile_pool(name="wl_ps", bufs=1, space="PSUM") as WP,
        ):
            def lstm_cell(gin, bias_sb, cstate, hout, n, nm):
                """Doubled-state LSTM cell: gin PSUM [B,GS] (+bias_sb),
                updates cstate (=2c) and hout (=2h, bf16) rows [0:n]."""
                if bias_sb is not None:
                    gs = W.tile([B, GS], f32, tag="gs" + nm)
                    nc.vector.tensor_tensor(out=gs[0:n, :], in0=gin[0:n, :],
                                            in1=bias_sb[0:n, :], op=OP.add)
                else:
                    gs = gin
                tg4 = W.tile([B, GS], f32, tag="tg4" + nm)
                nc.scalar.activation(tg4[0:n, :], gs[0:n, :], AT.Tanh, scale=0.5)
                ti = tg4[0:n, 0:HS]; tf = tg4[0:n, HS:2 * HS]
                to = tg4[0:n, 2 * HS:3 * HS]; tgg = tg4[0:n, 3 * HS:]
                aa = W.tile([B, HS], f32, tag="aa" + nm)
                nc.vector.scalar_tensor_tensor(
                    out=aa[0:n, :], in0=tf, scalar=1.0, in1=cstate[0:n, :],
                    op0=OP.add, op1=OP.mult)
                bb = W.tile([B, HS], f32, tag="bb" + nm)
                nc.vector.scalar_tensor_tensor(
                    out=bb[0:n, :], in0=ti, scalar=1.0, in1=tgg,
                    op0=OP.add, op1=OP.mult)
                nc.vector.scalar_tensor_tensor(
                    out=cstate[0:n, :], in0=aa[0:n, :], scalar=0.5, in1=bb[0:n, :],
                    op0=OP.mult, op1=OP.add)
                tc_ = W.tile([B, HS], f32, tag="tc" + nm)
                nc.scalar.activation(tc_[0:n, :], cstate[0:n, :], AT.Tanh, scale=0.5)
                nc.vector.scalar_tensor_tensor(
                    out=hout[0:n, :], in0=to, scalar=1.0, in1=tc_[0:n, :],
                    op0=OP.add, op1=OP.mult)

            def h_gather_start(hsrc, n, nm):
                """PE-transpose hsrc[0:n] -> bounce -> AllGather; returns ago."""
                tp = PSM.tile([128, B], bf16, tag="small", name="tp" + nm)
                nc.tensor.transpose(tp[:, 0:n], hsrc[:], ident[:, 0:n])
                hloc = W.tile([128, B], bf16, tag="hloc")
                nc.vector.tensor_copy(hloc[:, 0:n], tp[:, 0:n])
                agi = BP.tile([128, n], bf16, tag="agi" + nm)
                nc.sync.dma_start(agi[:], hloc[:, 0:n])
                ago = BP.tile([H1, n], bf16, tag="ago" + nm)
                nc.gpsimd.collective_compute("AllGather", OP.bypass, replica_groups=RG,
                                             ins=[agi.opt()], outs=[ago.opt()])
                return ago

            def h_gather(hsrc, hTdst, n, nm):
                ago = h_gather_start(hsrc, n, nm)
                nc.sync.dma_start(hTdst[:, :, 0:n],
                                  ago[:].rearrange("(k p) n -> p k n", p=128))

            # pre-loop: start g1 PSUM for step 0 (h1T is zeros; harmless MMs)
            g1ps = PG1.tile([B, GS], f32, tag="g1", name="g1_boot")
            for k in range(KT1):
                nc.tensor.matmul(g1ps[:], h1T[:, k, :], wh1[:, k, :],
                                 start=(k == 0), stop=False)

            for t in range(Tsteps):
                n = n_t[t]
                npr_t = (n + 1) // 2
                nnext = n_t[t + 1] if t + 1 < Tsteps else 0

                # ---- finish g1 = [Wh1@h1 (hoisted)] + W1h2@h2 ----
                for k in range(KT1):
                    nc.tensor.matmul(g1ps[0:n, :], h2T[:, k, 0:n], w1h2[:, k, :],
                                     start=False, stop=(k == KT1 - 1))
                # ---- g2 partial: Wh2@h2 (overlaps LSTM1 chain) ----
                g2ps = PG2.tile([B, GS], f32, tag="g2")
                for k in range(KT1):
                    nc.tensor.matmul(g2ps[0:n, :], h2T[:, k, 0:n], wh2[:, k, :],
                                     start=(k == 0), stop=False)

                # ---- LSTM1 cell ----
                lstm_cell(g1ps, pre_ring[t % 3], c1, h1n, n, "1")

                # ---- AllGather h1 ----
                h_gather(h1n, h1T, n, "1")

                # ---- attention: pps = (2h1)@(0.5 Wha.T) ----
                pps = PSM.tile([B, 512], f32, tag="small", name="pps")
                for k in range(KT1):
                    nc.tensor.matmul(pps[:, 0:PH], h1T[:, k, :], wha[:, k, :],
                                     start=(k == 0), stop=(k == KT1 - 1))
                # ---- g2 += W2h1@h1 ----
                for k in range(KT1):
                    nc.tensor.matmul(g2ps[0:n, :], h1T[:, k, 0:n], w2h1[:, k, :],
                                     start=False, stop=(k == KT1 - 1))
                # ---- hoist next step's Wh1@h1 into fresh g1 PSUM ----
                if t + 1 < Tsteps:
                    g1ps = PG1.tile([B, GS], f32, tag="g1", name=f"g1_{t+1}")
                    for k in range(KT1):
                        nc.tensor.matmul(g1ps[0:nnext, :], h1T[:, k, 0:nnext],
                                         wh1[:, k, :], start=(k == 0), stop=False)

                # ---- attention chain (vector/scalar) ----
                pbs = W.tile([B, PH], bf16, tag="pbs")
                nc.vector.tensor_copy(pbs[:], pps[:, 0:PH])
                vap = W.tile([B, RP, PH], bf16, tag="vap")
                nc.vector.tensor_tensor(out=vap[:], in0=Va[:],
                                        in1=pbs[:].unsqueeze(1).broadcast_to([B, RP, PH]), op=OP.add)
                tnh = W.tile([B, RP, PH], bf16, tag="tnh")
                nc.scalar.activation(tnh[:], vap[:], AT.Tanh)
                wprod = W.tile([B, RP, PH], bf16, tag="wprod")
                nc.vector.tensor_tensor(
                    out=wprod[:], in0=tnh[:],
                    in1=was[:].unsqueeze(1).broadcast_to([B, RP, PH]), op=OP.mult)
                logit = W.tile([B, RP], f32, tag="logit")
                nc.vector.tensor_reduce(logit[:], wprod[:], axis=mybir.AxisListType.X,
                                        op=OP.add)
                # transpose logits to [RP, B], apply pad mask, AllGather
                ltp = PSM.tile([RP, B], f32, tag="small", name="ltp")
                nc.tensor.transpose(ltp[:], logit[:], identf[:])
                lts = W.tile([RP, B], f32, tag="lts")
                nc.vector.tensor_scalar(out=lts[:], in0=ltp[:],
                                        scalar1=lmasks[:], scalar2=loffs[:],
                                        op0=OP.mult, op1=OP.add)
                agi2 = BP.tile([RP, n], f32, tag="agi2")
                nc.sync.dma_start(agi2[:], lts[:, 0:n])
                ago2 = BP.tile([RT, n], f32, tag="ago2")
                nc.gpsimd.collective_compute("AllGather", OP.bypass, replica_groups=RG,
                                             ins=[agi2.opt()], outs=[ago2.opt()])
                lall = W.tile([RT, B], f32, tag="lall")
                nc.sync.dma_start(lall[:, 0:n], ago2[:])

                # ---- vocab projection for step t-1 ----
                def vocab_chunk(tstep, v):
                    np_ = n_t[tstep]
                    v0, v1 = v * 512, min(VS, v * 512 + 512)
                    wps = WP.tile([128, 512], f32, tag="wps")
                    for k in range(KT1):
                        nc.tensor.matmul(wps[0:np_, 0:v1 - v0], h2T[:, k, 0:np_],
                                         wl[:, k, v0:v1],
                                         start=(k == 0), stop=(k == KT1 - 1))
                    ores = OW.tile([128, 512], f32, tag="ores")
                    nc.vector.tensor_tensor(out=ores[0:np_, 0:v1 - v0],
                                            in0=wps[0:np_, 0:v1 - v0],
                                            in1=bls[0:np_, v0:v1], op=OP.add)
                    nc.sync.dma_start(out.ap()[0:np_, tstep, v0:v1],
                                      ores[0:np_, 0:v1 - v0])

                if t >= 1:
                    for v in range(NVT - 1):       # chunks 0,1 in h1-AG window
                        vocab_chunk(t - 1, v)

                # ---- unnormalized softmax in [RT, B] layout ----
                esb = W.tile([RT, B], bf16, tag="esb")
                nc.scalar.activation(esb[:, 0:n], lall[:, 0:n], AT.Exp)
                ssum = PSM.tile([B, 4], f32, tag="small", name="ssum")
                nc.tensor.matmul(ssum[0:n, 0:1], esb[:, 0:n], ones_r[:],
                                 start=True, stop=True)
                sinv = W.tile([B, 1], f32, tag="sinv")
                nc.vector.reciprocal(sinv[0:n, :], ssum[0:n, 0:1])
                # scatter exp weights into block-diag lhsT (2 strided copies)
                for j in range(2):
                    cntj = (n - j + 1) // 2
                    if cntj > 0:
                        nc.vector.tensor_copy(
                            atten_bd[64 * j:64 * j + RT,
                                     j:j + 130 * (cntj - 1) + 1:130],
                            esb[:, j:j + 2 * (cntj - 1) + 1:2])

                # ---- einsum in two PSUM halves; merge half A while B runs ----
                nprA = (npr_t + 1) // 2
                nprB = npr_t - nprA
                psA = PSE.tile([B, GS], f32, tag="pseA")
                for i in range(nprA):
                    nc.tensor.matmul(psA[0:n, :],
                                     atten_bd[0:BDK, 128 * i:128 * i + n],
                                     VW2[0:BDK, i, :],
                                     start=(i == 0), stop=(i == nprA - 1))
                if nprB > 0:
                    psB = PSE.tile([B, GS], f32, tag="pseB")
                    for i in range(nprA, npr_t):
                        nc.tensor.matmul(psB[0:n, :],
                                         atten_bd[0:BDK, 128 * i:128 * i + n],
                                         VW2[0:BDK, i, :],
                                         start=(i == nprA), stop=(i == npr_t - 1))
                # gs2 = g2ps + (psA+psB)*sinv + b2, built so only one op
                # trails the einsum's last matmul
                gseA = W.tile([B, GS], f32, tag="gseA")
                nc.vector.scalar_tensor_tensor(
                    out=gseA[0:n, :], in0=psA[0:n, :], scalar=sinv[0:n, :],
                    in1=b2s[0:n, :], op0=OP.mult, op1=OP.add)
                if nprB > 0:
                    gsAB = W.tile([B, GS], f32, tag="gsAB")
                    nc.vector.tensor_tensor(out=gsAB[0:n, :], in0=g2ps[0:n, :],
                                            in1=gseA[0:n, :], op=OP.add)
                    gs2 = W.tile([B, GS], f32, tag="gs2f")
                    nc.vector.scalar_tensor_tensor(
                        out=gs2[0:n, :], in0=psB[0:n, :], scalar=sinv[0:n, :],
                        in1=gsAB[0:n, :], op0=OP.mult, op1=OP.add)
                    lstm_cell(gs2, None, c2, h2n, n, "2")
                else:
                    lstm_cell(g2ps, gseA, c2, h2n, n, "2")

                # ---- AllGather h2; deferred vocab chunk reads OLD h2T and
                # must be issued before the gathered h2 lands in h2T ----
                ago2h = h_gather_start(h2n, n, "2")
                if t >= 1:
                    vocab_chunk(t - 1, NVT - 1)
                if t + 2 < Tsteps:
                    emb_load(t + 2)
                    pre1_compute(t + 2, PSM, "small")
                nc.sync.dma_start(h2T[:, :, 0:n],
                                  ago2h[:].rearrange("(k p) n -> p k n", p=128))

            # ---- final step's vocab projection ----
            if Tsteps >= 1:
                np_ = n_t[Tsteps - 1]
                for v in range(NVT):
                    v0, v1 = v * 512, min(VS, v * 512 + 512)
                    wps = WP.tile([128, 512], f32, tag="wps")
                    for k in range(KT1):
                        nc.tensor.matmul(wps[0:np_, 0:v1 - v0], h2T[:, k, 0:np_],
                                         wl[:, k, v0:v1],
                                         start=(k == 0), stop=(k == KT1 - 1))
                    ores = OW.tile([128, 512], f32, tag="ores")
                    nc.vector.tensor_tensor(out=ores[0:np_, 0:v1 - v0],
                                            in0=wps[0:np_, 0:v1 - v0],
                                            in1=bls[0:np_, v0:v1], op=OP.add)
                    nc.sync.dma_start(out.ap()[0:np_, Tsteps - 1, v0:v1],
                                      ores[0:np_, 0:v1 - v0])

    nc.compile()
    return nc


def kernel(**inputs) -> np.ndarray:
    from concourse.bass_utils import run_bass_kernel_spmd

    in_maps, n_t = _prep_inputs(inputs)
    Tsteps = int(os.environ.get("K_TSTEPS", T))
    WvaT = np.ascontiguousarray(inputs["Wva"].T).astype(BF16)
    for m in in_maps:
        m["WvaT"] = WvaT

    key = (tuple(n_t), Tsteps)
    if key not in _cache:
        _cache[key] = _build(n_t, Tsteps)
    nc = _cache[key]

    global LAST_NC, LAST_IN_MAPS
    LAST_NC, LAST_IN_MAPS = nc, in_maps
    res = run_bass_kernel_spmd(nc, in_maps, core_ids=list(range(NC_)))
    outs = [res.results[c]["out"] for c in range(NC_)]
    full = np.concatenate(outs, axis=2).astype(np.float32)
    # zero out the t >= lengths[b] tail for steps the kernel never wrote
    if Tsteps < T:
        full[:, Tsteps:, :] = 0.0
    return full


if __name__ == "__main__":
    print("kernel module OK")

